# revision 12
# baseline (speedup 1.0000x reference)
"""CLVP attention kernel for 8 Trainium2 NeuronCores — v2 (bf16, group-pipelined).

Sharding: core c = 2*b + hg handles batch b and head-group hg (8 of 16 heads).
The 8 local heads are processed as 4 pair-groups (2 heads = 128 dims each).
Per group: q/k/v projection (bf16 matmuls, N=128 moving so single-pair groups
are full-rate), partial rotary on DVE, PE-transpose of q/k to [dims, tok],
then attention (scores transposed, exp on ScalarE, fused V|ones matmul whose
65th row is the softmax denominator).  Groups are software-pipelined: the
Tile scheduler overlaps group g+1's projections with group g's ACT-bound
attention stream, so the ScalarE exp stream (the 266us/core floor) starts
early and never starves.

Changes vs v1: everything bf16 on the PE (same 1 cycle/col as fp32r but no
N>=256 restriction and 2x faster weight loads), denominator broadcast via
GPSIMD partition_broadcast instead of PE matmuls (frees a PSUM bank + PE
cycles), softmax tail reads PSUM once then runs off SBUF so outA/outB banks
free early, out-projection deferred to group 3 via SBUF-resident o_norm
tiles, PSUM = exactly 8 banks (4 sps + outA + outB + 2 scratch).
"""

import numpy as np
import ml_dtypes

import concourse.bass as bass
import concourse.tile as tile
from concourse import bacc, mybir
from concourse.bass_utils import run_bass_kernel_spmd

B, S, E, H, D, ROT = 4, 2048, 1024, 16, 64, 32
HLOC = 8            # heads per core
HS = HLOC * D       # 512 head dims per core
G = 4               # pair-groups per core (2 heads each)
N_CORES = 8
KE = E // 128       # 8 contraction tiles for projections
TT = S // 128       # 16 token tiles
CH = 4              # token tiles per projection chunk
QC = S // 512       # 4 q chunks
KC = S // 128       # 16 k chunks

f32 = mybir.dt.float32
f32r = mybir.dt.float32r
bf16 = mybir.dt.bfloat16
fp8 = mybir.dt.float8e4
FT = mybir.ActivationFunctionType


def _emit(nc, tc, ctx, t):
    hidT, wq, wk, wv, m1, cmat, smat, ident_in, onesb_in, onesf_in, part = t
    w_dram = {"q": wq, "k": wk, "v": wv}

    const = ctx.enter_context(tc.tile_pool(name="const", bufs=1))
    ident = const.tile([128, 128], bf16)
    nc.sync.dma_start(ident[:], ident_in.ap())
    ones_f = const.tile([128, 64], f32r)
    nc.sync.dma_start(ones_f[:], onesf_in.ap())
    ones_b = const.tile([128, 1], bf16)
    nc.sync.dma_start(ones_b[:], onesb_in.ap())
    c_sb = const.tile([128, TT * 64], f32)
    nc.sync.dma_start(
        c_sb[:].rearrange("p (t d) -> p t d", d=64),
        cmat.ap().rearrange("(t p) d -> p t d", p=128),
    )
    s_sb = const.tile([128, TT * 32], f32)
    nc.sync.dma_start(
        s_sb[:].rearrange("p (t d) -> p t d", d=32),
        smat.ap().rearrange("(t p) d -> p t d", p=128),
    )
    # hidden in 8 per-emb-tile DMAs so the first projection starts early
    hid_sb = const.tile([128, KE * S], bf16)
    hid_v = hid_sb[:].rearrange("p (k s) -> p k s", s=S)
    nc.sync.dma_start(hid_v[:, 0:1, :], hidT.ap().rearrange("(k p) s -> p k s", p=128)[:, 0:1, :])
    w_sb = {}
    for name in ("k", "q", "v"):
        wt = const.tile([128, KE * HS], bf16, name=f"w{name}", tag=f"w{name}")
        nc.sync.dma_start(
            wt[:].rearrange("p (k n) -> p k n", n=HS),
            w_dram[name].ap().rearrange("(k p) n -> p k n", p=128),
        )
        w_sb[name] = wt
    for k in range(1, KE):
        nc.sync.dma_start(
            hid_v[:, k : k + 1, :],
            hidT.ap().rearrange("(k p) s -> p k s", p=128)[:, k : k + 1, :],
        )
    m1_sb = const.tile([128, G * E], bf16)
    nc.sync.dma_start(
        m1_sb[:].rearrange("p (t n) -> p t n", n=E),
        m1.ap().rearrange("(t p) n -> p t n", p=128),
    )

    qk_pool = ctx.enter_context(tc.tile_pool(name="qk", bufs=2))
    vext_pool = ctx.enter_context(tc.tile_pool(name="vex", bufs=2))
    xn_pool = ctx.enter_context(tc.tile_pool(name="xn", bufs=2))
    tmp_pool = ctx.enter_context(tc.tile_pool(name="tmp", bufs=4))
    ex_pool = ctx.enter_context(tc.tile_pool(name="exp", bufs=6))
    onorm_pool = ctx.enter_context(tc.tile_pool(name="onorm", bufs=16))
    oas_pool = ctx.enter_context(tc.tile_pool(name="oas", bufs=2))
    rz_pool = ctx.enter_context(tc.tile_pool(name="rz", bufs=2))
    ws_pool = ctx.enter_context(tc.tile_pool(name="ws", bufs=2))
    psum = ctx.enter_context(tc.tile_pool(name="psum", bufs=1, space="PSUM"))

    onorm_tiles = {}

    def emit_outproj(qc):
        for m in range(E // 128):
            wps = psum.tile([128, 512], f32, tag="scr", bufs=2, name="wps")
            for p in range(G):
                nc.tensor.matmul(
                    wps[:],
                    m1_sb[:, E * p + 128 * m : E * p + 128 * (m + 1)],
                    onorm_tiles[(p, qc)][:],
                    start=(p == 0),
                    stop=(p == G - 1),
                )
            ws = ws_pool.tile([128, 512], bf16, tag="ws")
            nc.vector.tensor_copy(ws[:], wps[:])
            nc.sync.dma_start(
                part.ap()[128 * m : 128 * (m + 1), 512 * qc : 512 * (qc + 1)], ws[:]
            )

    for g in range(G):
        # ---------------- projections + rotary + transpose for pair g ------
        qT = qk_pool.tile([128, S], bf16, tag="qT", name="qT")
        kT = qk_pool.tile([128, S], bf16, tag="kT", name="kT")
        vext = vext_pool.tile([128, KC * 2 * 65], bf16, tag="vex", name="vext")
        vext_v = vext[:].rearrange("p (kc h c) -> p kc h c", h=2, c=65)
        # ones column (softmax denominator row) via DVE free-axis broadcast
        nc.vector.tensor_copy(
            vext_v[:, :, :, 64:65],
            ones_b[:, 0:1]
            .rearrange("p (kc h c) -> p kc h c", kc=1, h=1)
            .broadcast_to([128, KC, 2, 1]),
        )

        for ch in range(TT // CH):
            xns = {}
            for X in ("k", "q", "v"):
                pp = psum.tile([128, 512], f32, tag="scr", bufs=2, name=f"pp{X}")
                for t2 in range(CH):
                    tt = CH * ch + t2
                    for k in range(KE):
                        nc.tensor.matmul(
                            pp[:, 128 * t2 : 128 * (t2 + 1)],
                            hid_v[:, k, 128 * tt : 128 * (tt + 1)],
                            w_sb[X][:, HS * k + 128 * g : HS * k + 128 * (g + 1)],
                            start=(k == 0),
                            stop=(k == KE - 1),
                        )
                # rotary on [tok, (4tt, 2h, 64d)] view
                psv = pp[:].rearrange("p (t h d) -> p t h d", h=2, d=64)
                if X == "v":
                    outv = vext_v[:, CH * ch : CH * (ch + 1), :, 0:64]
                else:
                    xn = xn_pool.tile([128, CH * 128], bf16, tag=f"x{X}", name=f"x{X}")
                    outv = xn[:].rearrange("p (t h d) -> p t h d", h=2, d=64)
                    xns[X] = xn
                cb = (
                    c_sb[:, 64 * CH * ch : 64 * CH * (ch + 1)]
                    .rearrange("p (t o d) -> p t o d", o=1, d=64)
                    .broadcast_to([128, CH, 2, 64])
                )
                nc.vector.tensor_mul(outv, psv, cb)
                tmp = tmp_pool.tile([128, CH * 64], bf16, tag="tmp")
                tmpv = tmp[:].rearrange("p (t h d) -> p t h d", h=2, d=32)
                s0 = (
                    s_sb[:, 32 * CH * ch : 32 * CH * (ch + 1)]
                    .rearrange("p (t o d) -> p t o d", o=1, d=32)[:, :, :, 0:16]
                    .broadcast_to([128, CH, 2, 16])
                )
                s1 = (
                    s_sb[:, 32 * CH * ch : 32 * CH * (ch + 1)]
                    .rearrange("p (t o d) -> p t o d", o=1, d=32)[:, :, :, 16:32]
                    .broadcast_to([128, CH, 2, 16])
                )
                nc.vector.tensor_mul(tmpv[:, :, :, 0:16], psv[:, :, :, 16:32], s0)
                nc.vector.tensor_mul(tmpv[:, :, :, 16:32], psv[:, :, :, 0:16], s1)
                rotslice = outv[:, :, :, 0:32]
                nc.vector.tensor_add(rotslice, rotslice, tmpv)
            # PE transposes of q/k into [dims, tok], batched 4 per PSUM tile
            for X in ("k", "q"):
                dest = kT if X == "k" else qT
                tr = psum.tile([128, 512], bf16, tag="scr", bufs=2, name="tr")
                for t2 in range(CH):
                    nc.tensor.transpose(
                        tr[:, 128 * t2 : 128 * (t2 + 1)],
                        xns[X][:, 128 * t2 : 128 * (t2 + 1)],
                        ident[:],
                    )
                nc.vector.tensor_copy(dest[:, 512 * ch : 512 * (ch + 1)], tr[:])

        # ---------------- attention for pair g ----------------------------
        for qc in range(QC):
            outA = psum.tile([65, 512], f32, tag="outA", bufs=1, name="outA")
            outB = psum.tile([65, 512], f32, tag="outB", bufs=1, name="outB")
            for kc in range(KC):
                sps = psum.tile([128, 1024], f32, tag="sps", bufs=2, name="sps")
                nc.tensor.matmul(
                    sps[:, 0:512],
                    kT[0:64, 128 * kc : 128 * (kc + 1)],
                    qT[0:64, 512 * qc : 512 * (qc + 1)],
                    start=True,
                    stop=True,
                    tile_position=(0, 0),
                )
                nc.tensor.matmul(
                    sps[:, 512:1024],
                    kT[64:128, 128 * kc : 128 * (kc + 1)],
                    qT[64:128, 512 * qc : 512 * (qc + 1)],
                    start=True,
                    stop=True,
                    tile_position=(64, 0),
                )
                ex = ex_pool.tile([128, 1024], bf16, tag="ex")
                nc.scalar.activation(ex[:], sps[:], FT.Exp)
                nc.tensor.matmul(
                    outA[:],
                    vext_v[:, kc, 0, :],
                    ex[:, 0:512],
                    start=(kc == 0),
                    stop=(kc == KC - 1),
                )
                nc.tensor.matmul(
                    outB[:],
                    vext_v[:, kc, 1, :],
                    ex[:, 512:1024],
                    start=(kc == 0),
                    stop=(kc == KC - 1),
                )
            # softmax tail: drain PSUM once, broadcast 1/z back into the
            # just-freed outA/outB banks via a K=1 matmul, normalize on DVE
            o_n = onorm_pool.tile([128, 512], bf16, tag="on", name="o_n")
            for hh, outps, otag in ((0, outA, "outA"), (1, outB, "outB")):
                oXs = oas_pool.tile([65, 512], f32, tag=f"o{hh}", name="oXs")
                nc.vector.tensor_copy(oXs[:], outps[:])
                rz = rz_pool.tile([65, 512], f32r, tag="rz", name="rz")
                with nc.allow_low_precision(reason="softmax denom recip"):
                    nc.vector.reciprocal(rz[64:65, :], oXs[64:65, :])
                zps = psum.tile([64, 512], f32, tag=otag, bufs=1, name="zps")
                nc.tensor.matmul(
                    zps[:],
                    ones_f[64:65, 0:64],
                    rz[64:65, :],
                    start=True,
                    stop=True,
                    tile_position=(64, 0),
                )
                nc.vector.tensor_mul(
                    o_n[64 * hh : 64 * (hh + 1), :], oXs[0:64, :], zps[:]
                )
            onorm_tiles[(g, qc)] = o_n
            if g == G - 1:
                emit_outproj(qc)


_NC_CACHE = {}


def _get_nc():
    if "nc" in _NC_CACHE:
        return _NC_CACHE["nc"]
    nc = bacc.Bacc("TRN2", target_bir_lowering=False, debug=False, num_devices=N_CORES)
    hidT = nc.dram_tensor("hidT", [E, S], bf16, kind="ExternalInput")
    wq = nc.dram_tensor("wq", [E, HS], bf16, kind="ExternalInput")
    wk = nc.dram_tensor("wk", [E, HS], bf16, kind="ExternalInput")
    wv = nc.dram_tensor("wv", [E, HS], bf16, kind="ExternalInput")
    m1 = nc.dram_tensor("m1", [HS, E], bf16, kind="ExternalInput")
    cmat = nc.dram_tensor("cmat", [S, 64], f32, kind="ExternalInput")
    smat = nc.dram_tensor("smat", [S, 32], f32, kind="ExternalInput")
    ident_in = nc.dram_tensor("ident", [128, 128], bf16, kind="ExternalInput")
    onesb_in = nc.dram_tensor("onesb", [128, 1], bf16, kind="ExternalInput")
    onesf_in = nc.dram_tensor("onesf", [128, 64], f32r, kind="ExternalInput")
    part = nc.dram_tensor("part", [E, S], bf16, kind="ExternalOutput")
    from contextlib import ExitStack

    with tile.TileContext(nc) as tc, ExitStack() as ctx:
        _emit(
            nc, tc, ctx,
            (hidT, wq, wk, wv, m1, cmat, smat, ident_in, onesb_in, onesf_in, part),
        )
    nc.compile()
    _NC_CACHE["nc"] = nc
    return nc


def _in_maps(hidden_states, rotary_pos_emb, Wq, Wk, Wv, Wo):
    scale = np.float32(D**-0.5)
    f = np.asarray(rotary_pos_emb, np.float32)[0]  # [S, ROT]
    cmat = np.ones((S, 64), np.float32)
    cmat[:, 0:ROT] = np.cos(f)
    smat = np.empty((S, ROT), np.float32)
    smat[:, 0:16] = -np.sin(f[:, 0:16])
    smat[:, 16:ROT] = np.sin(f[:, 16:ROT])
    ident = np.eye(128, dtype=ml_dtypes.bfloat16)
    onesb = np.ones((128, 1), ml_dtypes.bfloat16)
    onesf = np.ones((128, 64), np.float32)
    hs = np.asarray(hidden_states, np.float32)
    Wq, Wk, Wv, Wo = (np.asarray(w, np.float32) for w in (Wq, Wk, Wv, Wo))
    maps = []
    for c in range(N_CORES):
        b, hg = divmod(c, 2)
        rows = slice(hg * HS, (hg + 1) * HS)
        maps.append(
            {
                "hidT": np.ascontiguousarray(hs[b].T).astype(ml_dtypes.bfloat16),
                "wq": np.ascontiguousarray((Wq[rows] * scale).T).astype(ml_dtypes.bfloat16),
                "wk": np.ascontiguousarray(Wk[rows].T).astype(ml_dtypes.bfloat16),
                "wv": np.ascontiguousarray(Wv[rows].T).astype(ml_dtypes.bfloat16),
                "m1": np.ascontiguousarray(Wo[:, rows].T).astype(ml_dtypes.bfloat16),
                "cmat": cmat,
                "smat": smat,
                "ident": ident,
                "onesb": onesb,
                "onesf": onesf,
            }
        )
    return maps


def kernel(hidden_states, rotary_pos_emb, Wq, Wk, Wv, Wo, bo, _trace=False):
    nc = _get_nc()
    maps = _in_maps(hidden_states, rotary_pos_emb, Wq, Wk, Wv, Wo)
    res = run_bass_kernel_spmd(
        nc, maps, core_ids=list(range(N_CORES)), trace=_trace
    )
    out = np.empty((B, S, E), np.float32)
    bo = np.asarray(bo, np.float32)
    for b in range(B):
        p0 = np.asarray(res.results[2 * b]["part"], dtype=np.float32)
        p1 = np.asarray(res.results[2 * b + 1]["part"], dtype=np.float32)
        out[b] = (p0 + p1).T + bo
    if _trace:
        kernel._last_results = res
    return out
